# revision 26
# baseline (speedup 1.0000x reference)
"""Trainium2 Bass kernel for a pre-LN transformer block (B=4, S=2048, H=12, D=64).

Sharding: 8 cores; core c -> batch b = c//2, parity p = c%2.
Each core handles the 1024 query rows of its batch whose 128-token block index
has parity p (stride-2 interleave balances causal load; SPMD shared program).

Host-side layout (from v2): the host PERMUTES each core's token axis so that
the core's own query tokens are columns 0:1024 and the other parity's tokens
are columns 1024:2048.  Q projection / residual / output then just use the
first half of the feature-major activations; causal structure is carried
entirely by per-core host-computed multiplicative masks.

v3 changes vs v2 (HAM-warmth + engine-balance pass):
- LayerNorm: squares on ACT, stats math on a single partition lane, a/c
  broadcast across partitions on GpSimd (partition_broadcast) -> no PE
  broadcast matmuls, less DVE work, fewer PSUM tiles.
- Softmax normalization: reciprocal of the ones-row denominator straight from
  PSUM (DVE), partition_broadcast on GpSimd, then one DVE multiply per head
  reading O from PSUM.  No CASTs, no expand matmuls -> PSUM banks free fast,
  PE never idles >3.4us at head-pair boundaries (HAM stays at full clock).
- span>512 score tiles split into two merged head-pair chunks (finer PSUM
  rotation, single exp per chunk, double-buffered throughout).
- Mask multiplies merged: one DVE op covers both heads' diagonal blocks
  (mask tensor host-duplicated per head).
- Wo fully resident, token-chunk-outer with fused residual -> no cold
  restart between attention and MLP.
"""

import numpy as np

N_CORES = 8
B, S, H, D = 4, 2048, 12, 64
HID = 768
QL = 1024
KT = HID // 128     # 6 feature blocks
TT = S // 128       # 16 key tiles
MH = 4 * HID // 128  # 24 hidden blocks
EPS = 1e-5

_CACHE = {}


def _build_program(biases_zero, debug=False):
    from contextlib import ExitStack
    import concourse.bass as bass
    import concourse.tile as tile
    from concourse import bacc, mybir

    F32 = mybir.dt.float32
    BF16 = mybir.dt.bfloat16
    Alu = mybir.AluOpType
    Act = mybir.ActivationFunctionType

    nc = bacc.Bacc("TRN2", target_bir_lowering=False, debug=False,
                   enable_asserts=False, num_devices=N_CORES)

    def din(name, shape, dt):
        return nc.dram_tensor(name, shape, dt, kind="ExternalInput").ap()

    # activations / consts (per-core)
    xbT = din("xbT", [128, KT, S], BF16)          # packed, token-permuted
    masks = din("masks", [128, 2, 2, 128], BF16)  # [tri|flat] x [head-rep]
    # weights (shared across cores), host-packed per-partition-contiguous
    Wq = din("Wq", [128, KT, HID], BF16)       # [p][kt][m]; ln1_w and 1/8 folded
    Wk = din("Wk", [128, KT, HID], BF16)       # ln1_w folded
    Wv = din("Wv", [128, KT, HID], BF16)       # ln1_w folded
    Wo = din("Wo", [128, KT, HID], BF16)
    W1 = din("W1", [MH, 128, KT, 128], BF16)   # [mo][p][kt][cols]; ln2_w folded
    W2 = din("W2", [MH, 128, HID], BF16)       # [k2][p][m]
    bqs = din("bqs", [HID], F32)
    bk = din("bk", [HID], F32)
    bv = din("bv", [HID], F32)
    bo = din("bo", [HID], F32)
    b1 = din("b1", [4 * HID], F32)
    b2 = din("b2", [HID], F32)

    y = nc.dram_tensor("y", [HID, QL], F32, kind="ExternalOutput").ap()
    if debug:
        d_ln1 = nc.dram_tensor("d_ln1", [128, KT, S], BF16,
                               kind="ExternalOutput").ap()
        d_K = nc.dram_tensor("d_K", [128, KT, S], BF16,
                             kind="ExternalOutput").ap()
        d_attn = nc.dram_tensor("d_attn", [128, KT, QL], BF16,
                                kind="ExternalOutput").ap()
        d_r = nc.dram_tensor("d_r", [128, KT, QL], BF16,
                             kind="ExternalOutput").ap()
        d_ln2 = nc.dram_tensor("d_ln2", [128, KT, QL], BF16,
                               kind="ExternalOutput").ap()

    def bcast(src_elem_ap, parts, n):
        return bass.AP(tensor=src_elem_ap.tensor, offset=src_elem_ap.offset,
                       ap=[[0, parts], [1, n]])

    from concourse import library_config

    with tile.TileContext(nc) as tc, ExitStack() as ctx:
        sb = ctx.enter_context(tc.tile_pool(name="sb", bufs=1))
        ps = ctx.enter_context(tc.tile_pool(name="ps", bufs=1, space="PSUM"))

        # partition_broadcast lives in the gpsimd "attn" ucode library
        nc.gpsimd.load_library(library_config.attn)

        def pst_tile(name, shape=(128, 1024)):
            return ps.tile(list(shape), F32, tag="s2", bufs=4, name=name,
                           padded_shape=[128, 1024])

        # ---------- constants ----------
        ones_bf = sb.tile([128, 1], BF16, tag="ones")
        nc.vector.memset(ones_bf, 1.0)
        par = sb.tile([128, 80], F32, tag="par")

        def load_cols(dst0, src, n):
            nc.gpsimd.dma_start(
                out=par[:, dst0:dst0 + n],
                in_=bass.AP(tensor=src.tensor, offset=src.offset,
                            ap=[[1, 128], [128, n]]))

        load_cols(24, bqs, KT)
        load_cols(30, bk, KT)
        load_cols(36, bo, KT)
        load_cols(42, b2, KT)
        load_cols(48, b1, MH)
        nc.vector.memset(par[:, 72:73], EPS)
        eps_t = par[0:1, 72:73]
        if not biases_zero:
            bv_b = sb.tile([128, HID], F32, tag="bv_b")
            nc.gpsimd.dma_start(out=bv_b, in_=bcast(bv[0], 128, HID))
        masks_sb = sb.tile([128, 2, 2, 128], BF16, tag="masks")
        nc.gpsimd.dma_start(out=masks_sb, in_=masks)

        # PE warmup: lift the HAM clock gate while input DMAs land
        warm = sb.tile([128, 512], BF16, tag="warm", bufs=1, name="warm")
        nc.vector.memset(warm, 0.0)
        wps = pst_tile("warmps", (1, 512))
        for i in range(8):
            nc.tensor.matmul(wps, ones_bf, warm, start=True, stop=True)

        # ---------- input load ----------
        xbT_sb = sb.tile([128, KT, S], BF16, tag="xbT")
        for c in range(S // 512):
            nc.sync.dma_start(out=xbT_sb[:, :, 512 * c:512 * c + 512],
                              in_=xbT[:, :, 512 * c:512 * c + 512])

        # ---------- LN helper: 1-lane stats + gpsimd partition broadcast ----
        def emit_ln(N, x_bf, out_bf, pfx):
            # 1-lane stats scratch shares the "lane" tag with attention's rr
            lnst = sb.tile([1, 4, 512], F32, tag="lane", bufs=1,
                           name=f"{pfx}lnst")
            for c in range(N // 512):
                off = 512 * c
                cs = slice(off, off + 512)
                s_ps = pst_tile(f"{pfx}s{c}", (1, 512))
                q_ps = pst_tile(f"{pfx}q{c}", (1, 512))
                for kt in range(KT):
                    nc.tensor.matmul(s_ps, ones_bf, x_bf[:, kt, cs],
                                     start=(kt == 0), stop=(kt == KT - 1))
                for kt in range(KT):
                    sqc = sb.tile([128, 512], BF16, tag="sq", bufs=2,
                                  name=f"{pfx}sq{c}_{kt}")
                    nc.scalar.activation(sqc, x_bf[:, kt, cs], Act.Square)
                    nc.tensor.matmul(q_ps, ones_bf, sqc,
                                     start=(kt == 0), stop=(kt == KT - 1))
                # 1-lane stats math: mu, m2, var, a32 (rstd)
                mu, m2, var, a32 = (lnst[:, i, :] for i in range(4))
                nc.vector.tensor_scalar_mul(mu, s_ps, 1.0 / HID)
                nc.vector.tensor_mul(m2, mu, mu)
                nc.vector.scalar_tensor_tensor(var, q_ps, 1.0 / HID, m2,
                                               Alu.mult, Alu.subtract)
                sd = m2  # reuse slot
                nc.scalar.activation(sd, var, Act.Sqrt, bias=eps_t, scale=1.0)
                nc.vector.reciprocal_approx_fast(out=a32, in_=sd)
                ac = sb.tile([1, 2, 512], BF16, tag="lnac", bufs=2,
                             name=f"{pfx}ac{c}")
                nc.vector.tensor_copy(ac[:, 0, :], a32)
                nc.vector.scalar_tensor_tensor(ac[:, 1, :], a32, -1.0, mu,
                                               Alu.mult, Alu.mult)  # -mu*rstd
                # broadcast across partitions on GpSimd
                acb = sb.tile([128, 2, 512], BF16, tag="lnacb", bufs=2,
                              name=f"{pfx}acb{c}")
                nc.gpsimd.partition_broadcast(acb[:, 0, :], ac[:, 0, :],
                                              channels=128)
                nc.gpsimd.partition_broadcast(acb[:, 1, :], ac[:, 1, :],
                                              channels=128)
                # apply: out = x*a + c  (bf16, 2x DVE mode)
                for kt in range(KT):
                    t0 = sb.tile([128, 512], BF16, tag="t0", bufs=2,
                                 name=f"{pfx}t0{c}_{kt}")
                    nc.vector.tensor_mul(t0, x_bf[:, kt, cs], acb[:, 0, :])
                    nc.vector.tensor_add(out_bf[:, kt, cs], t0, acb[:, 1, :])

        ln_bf = sb.tile([128, KT, S], BF16, tag="ln")
        emit_ln(S, xbT_sb, ln_bf, "l1")
        if debug:
            nc.scalar.dma_start(out=d_ln1, in_=ln_bf)

        # evacuation helper: psum -> sbuf (+ optional bias col)
        def evac(dst, src, bias_col=None):
            if biases_zero or bias_col is None:
                nc.scalar.copy(dst, src)
            else:
                nc.vector.tensor_scalar(dst, src, 1.0, bias_col,
                                        Alu.mult, Alu.add)

        # ---------- QKV projections (mo-outer, weights loaded once) ----------
        K_sb = sb.tile([128, KT, S], BF16, tag="K")
        Q_sb = sb.tile([128, KT, QL], BF16, tag="Q")
        # n-outer so chunk c's K matmuls start as soon as LN1 chunk c is
        # applied (keeps the PE warm through LN1); weights on the gpsimd
        # queue so they don't serialize behind the x input DMAs
        for n in range(S // 512):
            cs = slice(512 * n, 512 * n + 512)
            for mo in range(KT):
                wkt = sb.tile([128, KT, 128], BF16, tag="wk6", bufs=2,
                              name=f"wk{n}_{mo}")
                nc.gpsimd.dma_start(out=wkt,
                                    in_=Wk[:, :, 128 * mo:128 * mo + 128])
                pst = pst_tile(f"kps{mo}_{n}", (128, 512))
                for kt in range(KT):
                    nc.tensor.matmul(pst, wkt[:, kt, :], ln_bf[:, kt, cs],
                                     start=(kt == 0), stop=(kt == KT - 1))
                evac(K_sb[:, mo, cs], pst, par[:, 30 + mo:31 + mo])
        for mo in range(KT):
            wqt = sb.tile([128, KT, 128], BF16, tag="wk6", bufs=2,
                          name=f"wq{mo}")
            nc.sync.dma_start(out=wqt, in_=Wq[:, :, 128 * mo:128 * mo + 128])
            for n in range(QL // 512):
                cs = slice(512 * n, 512 * n + 512)
                pst = pst_tile(f"qps{mo}_{n}", (128, 512))
                for kt in range(KT):
                    nc.tensor.matmul(pst, wqt[:, kt, :], ln_bf[:, kt, cs],
                                     start=(kt == 0), stop=(kt == KT - 1))
                evac(Q_sb[:, mo, cs], pst, par[:, 24 + mo:25 + mo])

        V_sb = sb.tile([128, H, TT, 65], BF16, tag="V")
        for h in range(H):
            nc.vector.memset(V_sb[:, h, :, 64:65], 1.0)
        for fc in range(2):
            wvt = sb.tile([128, KT, 384], BF16, tag="wv", bufs=1, name=f"wv{fc}")
            nc.sync.dma_start(out=wvt,
                              in_=Wv[:, :, 384 * fc:384 * fc + 384])
            for tt in range(TT):
                pst = pst_tile(f"vps{tt}_{fc}", (128, 384))
                for kt in range(KT):
                    nc.tensor.matmul(pst, ln_bf[:, kt, 128 * tt:128 * tt + 128],
                                     wvt[:, kt, :],
                                     start=(kt == 0), stop=(kt == KT - 1))
                vdst = V_sb[:, 6 * fc:6 * fc + 6, tt, 0:64]
                if biases_zero:
                    nc.scalar.copy(vdst, pst.rearrange("p (h d) -> p h d", d=64))
                else:
                    nc.vector.tensor_tensor(
                        vdst, pst.rearrange("p (h d) -> p h d", d=64),
                        bv_b[:, 384 * fc:384 * fc + 384].rearrange(
                            "p (h d) -> p h d", d=64),
                        Alu.add)

        # prefetch Wo + a third of W1 during attention (gpsimd queue); the
        # remaining W1 streams per-mo during the MLP
        Wo_sb = sb.tile([128, KT, HID], BF16, tag="Wo")
        nc.gpsimd.dma_start(out=Wo_sb, in_=Wo)
        W1_sb = sb.tile([128, 8, KT, 128], BF16, tag="W1")
        for mo in range(8):
            nc.gpsimd.dma_start(out=W1_sb[:, mo, :, :], in_=W1[mo])

        # ---------- attention ----------
        attn_bf = sb.tile([128, KT, QL], BF16, tag="attn")
        # denominator reciprocal rows live at partition 64 (DVE lanes are
        # partition-aligned; O's ones-row is at partition 64)
        rr = sb.tile([65, 2, QL], F32, tag="lane", bufs=1, name="rr")
        # full 128 partitions: broadcast ucode may write all partitions of
        # the column range regardless of channels; give it its own rows
        rb = sb.tile([128, 2, QL], F32, tag="rbc", bufs=1, name="rb")
        for kt in range(KT):
            h0, h1 = 2 * kt, 2 * kt + 1
            O = {h0: pst_tile(f"o{h0}", (65, QL)),
                 h1: pst_tile(f"o{h1}", (65, QL))}

            def av(h, tile_t, e, c0, w):
                nc.tensor.matmul(O[h][:, c0:c0 + w],
                                 V_sb[:, h, tile_t, :], e,
                                 start=(tile_t == 0),
                                 stop=(tile_t == TT - 1),
                                 skip_group_check=True)

            prev = []
            for t in range(TT):
                q0 = 128 * (t % 8)
                midx = 0 if t < 8 else 1
                # chunks of the query span [q0:QL), each <=512 wide; merged
                # head-pair scores: h0 at [0:w], h1 at [512:512+w]
                if q0 < 512:
                    chunks = [(q0, 512 - q0, True), (512, 512, False)]
                else:
                    chunks = [(q0, QL - q0, True)]
                for (c0, w, diag) in chunks:
                    # h0 right-aligned at [512-w:512], h1 at [512:512+w] so
                    # the exp covers one contiguous span with no gap
                    S_m = pst_tile(f"sm{kt}_{t}_{c0}", (128, 1024))
                    for pr, co in ((slice(0, 64), 512 - w),
                                   (slice(64, 128), 512)):
                        nc.tensor.matmul(S_m[:, co:co + w],
                                         K_sb[pr, kt, 128 * t:128 * t + 128],
                                         Q_sb[pr, kt, c0:c0 + w],
                                         start=True, stop=True)
                    eS = sb.tile([128, 1024], BF16, tag="expS", bufs=3,
                                 name=f"es{kt}_{t}_{c0}")
                    nc.scalar.activation(eS[:, 512 - w:512 + w],
                                         S_m[:, 512 - w:512 + w], Act.Exp)
                    if diag:
                        for co in (512 - w, 512):
                            nc.vector.tensor_mul(
                                eS[:, co:co + 128], eS[:, co:co + 128],
                                masks_sb[:, midx, 0, :])
                    cur = [(h0, eS[:, 512 - w:512], c0, w, t),
                           (h1, eS[:, 512:512 + w], c0, w, t)]
                    for (h, e, pc0, pw, pt) in prev:
                        av(h, pt, e, pc0, pw)
                    prev = cur
            for (h, e, pc0, pw, pt) in prev:
                av(h, pt, e, pc0, pw)

            # Evacuate each head's full O (values + ones-row denominator) to
            # SBUF in ONE DVE op -> PSUM banks free 1.2us after the last AV.
            # Then: DMA the denominator row to partition 0, broadcast the raw
            # denominator on GpSimd (ucode reads partition 0 only), take the
            # reciprocal in place at partitions 0:64 (custom-DVE is broken at
            # non-zero base), and normalize from SBUF.
            nc.vector.tensor_copy(rr[0:65, 0, :], O[h0][0:65, :])
            nc.vector.tensor_copy(rr[0:65, 1, :], O[h1][0:65, :])
            for i in range(2):
                nc.sync.dma_start(out=rb[0:1, i, :], in_=rr[64:65, i, :])
                nc.gpsimd.partition_broadcast(rb[:, i, :], rb[0:1, i, :],
                                              channels=128)
                nc.vector.reciprocal_approx_fast(out=rb[0:64, i, :],
                                                 in_=rb[0:64, i, :])
            nc.vector.tensor_mul(attn_bf[0:64, kt, :], rr[0:64, 0, :],
                                 rb[0:64, 0, :])
            stg = sb.tile([64, QL], BF16, tag="stg", bufs=1, name=f"stg{kt}")
            nc.vector.tensor_mul(stg, rr[0:64, 1, :], rb[0:64, 1, :])
            nc.sync.dma_start(out=attn_bf[64:128, kt, :], in_=stg)

        # prefetch W2 (gpsimd queue) into space freed by Q
        if debug:
            nc.scalar.dma_start(out=d_K, in_=K_sb)
            nc.scalar.dma_start(out=d_attn, in_=attn_bf)
        W2a = sb.tile([128, 8, HID], BF16, tag="Q", name="W2a")
        for k2 in range(8):
            nc.gpsimd.dma_start(out=W2a[:, k2, :], in_=W2[k2])

        # ---------- Wo + residual (bf16), token-chunk-outer, streamed Wo ----
        r_bf = sb.tile([128, KT, QL], BF16, tag="r")
        for n in range(QL // 512):
            cs = slice(512 * n, 512 * n + 512)
            for mo in range(KT):
                pst = pst_tile(f"ops{mo}_{n}", (128, 512))
                for kt in range(KT):
                    nc.tensor.matmul(pst,
                                     Wo_sb[:, kt, 128 * mo:128 * mo + 128],
                                     attn_bf[:, kt, cs],
                                     start=(kt == 0), stop=(kt == KT - 1))
                if biases_zero:
                    nc.vector.tensor_add(r_bf[:, mo, cs], pst,
                                         xbT_sb[:, mo, cs])
                else:
                    nc.vector.scalar_tensor_tensor(r_bf[:, mo, cs], pst,
                                                   par[:, 36 + mo:37 + mo],
                                                   xbT_sb[:, mo, cs],
                                                   Alu.add, Alu.add)

        # prefetch rest of W2 into space freed by attn / xbT
        if debug:
            nc.scalar.dma_start(out=d_r, in_=r_bf)
        W2b = sb.tile([128, 8, HID], BF16, tag="attn", name="W2b")
        for k2 in range(8):
            nc.gpsimd.dma_start(out=W2b[:, k2, :], in_=W2[8 + k2])
        W2c = sb.tile([128, 8, HID], BF16, tag="xbT", name="W2c")
        for k2 in range(8):
            nc.gpsimd.dma_start(out=W2c[:, k2, :], in_=W2[16 + k2])
        W2t = (W2a, W2b, W2c)

        ln2_bf = sb.tile([128, KT, QL], BF16, tag="V", name="ln2")
        emit_ln(QL, r_bf, ln2_bf, "l2")

        # ---------- MLP: W1 mo-outer (half resident), then W2 per cs-half ----
        if debug:
            nc.scalar.dma_start(out=d_ln2, in_=ln2_bf)
        # keep-warm: dummy matmuls tied to LN2 chunk-0 slices keep the PE
        # clock hot across the LN2 latency bubble before the MLP
        wps2 = pst_tile("warm2", (1, 512))
        for kt in range(KT):
            nc.tensor.matmul(wps2, ones_bf, ln2_bf[:, kt, 0:512],
                             start=True, stop=True)
        g0 = sb.tile([128, 12, QL], BF16, tag="ln", name="g0")
        g1 = sb.tile([128, 12, QL], BF16, tag="K", name="g1")
        for mo in range(MH):
            gt = g0 if mo < 12 else g1
            if mo < 8:
                w1t = W1_sb[:, mo, :, :]
            else:
                w1t = sb.tile([128, KT, 128], BF16, tag="wk6", bufs=2,
                              name=f"w1s{mo}")
                nc.gpsimd.dma_start(out=w1t, in_=W1[mo])
            for n in range(QL // 512):
                cs = slice(512 * n, 512 * n + 512)
                pst = pst_tile(f"h1ps{mo}_{n}", (128, 512))
                for kt in range(KT):
                    nc.tensor.matmul(pst, w1t[:, kt, :],
                                     ln2_bf[:, kt, cs],
                                     start=(kt == 0), stop=(kt == KT - 1))
                if biases_zero:
                    nc.scalar.activation(gt[:, mo % 12, cs], pst, Act.Gelu)
                else:
                    nc.scalar.activation(gt[:, mo % 12, cs], pst, Act.Gelu,
                                         bias=par[:, 48 + mo:49 + mo],
                                         scale=1.0)
        for n in range(QL // 512):
            cs = slice(512 * n, 512 * n + 512)
            psts = [pst_tile(f"yps{n}_{i}") for i in range(3)]
            for k2 in range(MH):
                gt = g0 if k2 < 12 else g1
                for mo in range(KT):
                    nc.tensor.matmul(
                        psts[mo // 2][:, 512 * (mo % 2):512 * (mo % 2) + 512],
                        W2t[k2 // 8][:, k2 % 8, 128 * mo:128 * mo + 128],
                        gt[:, k2 % 12, cs],
                        start=(k2 == 0), stop=(k2 == MH - 1))
            for mo in range(KT):
                pslice = psts[mo // 2][:, 512 * (mo % 2):512 * (mo % 2) + 512]
                yst = sb.tile([128, 512], F32, tag="yst", bufs=2,
                              name=f"yst{n}_{mo}")
                if biases_zero:
                    nc.vector.tensor_add(yst, pslice, r_bf[:, mo, cs])
                else:
                    nc.vector.scalar_tensor_tensor(yst, pslice,
                                                   par[:, 42 + mo:43 + mo],
                                                   r_bf[:, mo, cs],
                                                   Alu.add, Alu.add)
                nc.sync.dma_start(out=y[128 * mo:128 * mo + 128, cs], in_=yst)

    nc.compile()
    return nc


def _get_program(biases_zero):
    key = ("nc", biases_zero)
    if key not in _CACHE:
        _CACHE[key] = _build_program(biases_zero)
    return _CACHE[key]


def _prep_in_maps(inputs):
    import ml_dtypes
    bf = ml_dtypes.bfloat16
    f32 = np.float32

    x = np.ascontiguousarray(np.asarray(inputs["x"], dtype=f32))
    ln1w = np.asarray(inputs["ln1_w"], f32)
    ln1b = np.asarray(inputs["ln1_b"], f32)
    ln2w = np.asarray(inputs["ln2_w"], f32)
    ln2b = np.asarray(inputs["ln2_b"], f32)
    Wq = np.asarray(inputs["Wq"], f32)
    Wk = np.asarray(inputs["Wk"], f32)
    Wv = np.asarray(inputs["Wv"], f32)
    Wo = np.asarray(inputs["Wo"], f32)
    W1 = np.asarray(inputs["W1"], f32)
    W2 = np.asarray(inputs["W2"], f32)
    # fold LN gains into consumer weights; LN bias contribution into proj
    # biases; fold the 1/sqrt(D) score scale into Wq
    Wq_f = ln1w[:, None] * Wq * np.float32(1.0 / np.sqrt(D))
    Wk_f = ln1w[:, None] * Wk
    Wv_f = ln1w[:, None] * Wv
    W1_f = ln2w[:, None] * W1
    bq_e = (Wq.T @ ln1b + np.asarray(inputs["bq"], f32)) / np.float32(np.sqrt(D))
    bk_e = Wk.T @ ln1b + np.asarray(inputs["bk"], f32)
    bv_e = Wv.T @ ln1b + np.asarray(inputs["bv"], f32)
    b1_e = W1.T @ ln2b + np.asarray(inputs["b1"], f32)
    bo_e = np.asarray(inputs["bo"], f32)
    b2_e = np.asarray(inputs["b2"], f32)
    biases_zero = bool(
        all(np.all(v == 0) for v in (bq_e, bk_e, bv_e, b1_e, bo_e, b2_e)))

    def pack_kp(W):  # [HID, M] -> [128, KT, M] with row k*128+p -> [p, k]
        M = W.shape[1]
        return np.ascontiguousarray(
            W.reshape(KT, 128, M).transpose(1, 0, 2).astype(bf))

    W1p = pack_kp(W1_f)                       # [128, KT, 3072]
    W1p = np.ascontiguousarray(
        W1p.reshape(128, KT, MH, 128).transpose(2, 0, 1, 3))  # [mo][p][kt][128]
    W2p = np.ascontiguousarray(
        W2.reshape(MH, 128, HID).astype(bf))  # [k2][p][m]

    shared = {
        "Wq": pack_kp(Wq_f),
        "Wk": pack_kp(Wk_f),
        "Wv": pack_kp(Wv_f),
        "Wo": pack_kp(Wo),
        "W1": W1p,
        "W2": W2p,
        "bqs": bq_e, "bk": bk_e, "bv": bv_e, "bo": bo_e,
        "b1": b1_e, "b2": b2_e,
    }

    in_maps = []
    qcols_all = []
    for c in range(N_CORES):
        b, p = c // 2, c % 2
        qcols = np.concatenate(
            [np.arange(128 * (2 * j + p), 128 * (2 * j + p) + 128)
             for j in range(8)])
        ocols = np.concatenate(
            [np.arange(128 * (2 * j + 1 - p), 128 * (2 * j + 1 - p) + 128)
             for j in range(8)])
        qcols_all.append(qcols)
        xp = np.concatenate([x[b][qcols], x[b][ocols]], axis=0)  # [S, HID]
        xbT = np.ascontiguousarray(
            xp.T.reshape(KT, 128, S).transpose(1, 0, 2).astype(bf))
        # masks: [tri (own-parity diagonal) | flat (other-parity first block)]
        # duplicated along a head-rep axis so one DVE op masks both heads
        m = np.zeros((2, 128, 128), np.float32)
        kk = np.arange(128)[:, None]
        qq = np.arange(128)[None, :]
        m[0] = (kk <= qq).astype(np.float32)
        m[1] = 0.0 if p == 0 else 1.0
        m2 = np.stack([m, m], axis=1)                   # [2, 2, 128, 128]
        mperm = np.ascontiguousarray(m2.transpose(2, 0, 1, 3))  # [128,2,2,128]
        im = dict(shared)
        im["xbT"] = xbT
        im["masks"] = mperm.astype(bf)
        in_maps.append(im)
    return in_maps, qcols_all, biases_zero


def kernel(**inputs):
    import sys, types
    if "antenv.axon_hooks" not in sys.modules:
        try:
            sys.path.insert(0, "/root/.axon_site")
            from trn_agent_boot.trn_boot import _ntff_profile_via_ctypes
            hook = _ntff_profile_via_ctypes("/opt/axon/libaxon_pjrt.so")
            mod = types.ModuleType("antenv.axon_hooks")
            mod.get_axon_ntff_profile_hook = lambda: hook
            mod.set_axon_ntff_profile_hook = lambda h: None
            import antenv  # noqa: F401
            sys.modules["antenv.axon_hooks"] = mod
        except Exception:
            pass

    from concourse.bass_utils import run_bass_kernel_spmd

    in_maps, qcols_all, biases_zero = _prep_in_maps(inputs)
    nc = _get_program(biases_zero)
    res = run_bass_kernel_spmd(nc, in_maps, core_ids=list(range(N_CORES)))
    out = np.zeros((B, S, HID), np.float32)
    for c in range(N_CORES):
        out[c // 2, qcols_all[c], :] = res.results[c]["y"].T
    return out


# revision 28
# speedup vs baseline: 1.1522x; 1.1522x over previous
"""Trainium2 Bass kernel for a pre-LN transformer block (B=4, S=2048, H=12, D=64).

Sharding: 8 cores; core c -> batch b = c//2, parity p = c%2.
Each core handles the 1024 query rows of its batch whose 128-token block index
has parity p (stride-2 interleave balances causal load; SPMD shared program).

Host-side layout (from v2): the host PERMUTES each core's token axis so that
the core's own query tokens are columns 0:1024 and the other parity's tokens
are columns 1024:2048.  Q projection / residual / output then just use the
first half of the feature-major activations; causal structure is carried
entirely by per-core host-computed multiplicative masks.

v3 changes vs v2 (HAM-warmth + engine-balance pass):
- LayerNorm: squares on ACT, stats math on a single partition lane, a/c
  broadcast across partitions on GpSimd (partition_broadcast) -> no PE
  broadcast matmuls, less DVE work, fewer PSUM tiles.
- Softmax normalization: reciprocal of the ones-row denominator straight from
  PSUM (DVE), partition_broadcast on GpSimd, then one DVE multiply per head
  reading O from PSUM.  No CASTs, no expand matmuls -> PSUM banks free fast,
  PE never idles >3.4us at head-pair boundaries (HAM stays at full clock).
- span>512 score tiles split into two merged head-pair chunks (finer PSUM
  rotation, single exp per chunk, double-buffered throughout).
- Mask multiplies merged: one DVE op covers both heads' diagonal blocks
  (mask tensor host-duplicated per head).
- Wo fully resident, token-chunk-outer with fused residual -> no cold
  restart between attention and MLP.
"""

import numpy as np

N_CORES = 8
B, S, H, D = 4, 2048, 12, 64
HID = 768
QL = 1024
KT = HID // 128     # 6 feature blocks
TT = S // 128       # 16 key tiles
MH = 4 * HID // 128  # 24 hidden blocks
EPS = 1e-5

_CACHE = {}


def _build_program(biases_zero, debug=False):
    from contextlib import ExitStack
    import concourse.bass as bass
    import concourse.tile as tile
    from concourse import bacc, mybir

    F32 = mybir.dt.float32
    BF16 = mybir.dt.bfloat16
    Alu = mybir.AluOpType
    Act = mybir.ActivationFunctionType

    nc = bacc.Bacc("TRN2", target_bir_lowering=False, debug=False,
                   enable_asserts=False, num_devices=N_CORES)

    def din(name, shape, dt):
        return nc.dram_tensor(name, shape, dt, kind="ExternalInput").ap()

    # activations / consts (per-core)
    xbT = din("xbT", [128, KT, S], BF16)          # packed, token-permuted
    masks = din("masks", [128, 2, 2, 128], BF16)  # [tri|flat] x [head-rep]
    # weights (shared across cores), host-packed per-partition-contiguous
    Wq = din("Wq", [128, KT, HID], BF16)       # [p][kt][m]; ln1_w and 1/8 folded
    Wk = din("Wk", [128, KT, HID], BF16)       # ln1_w folded
    Wv = din("Wv", [128, KT, HID], BF16)       # ln1_w folded
    Wo = din("Wo", [128, KT, HID], BF16)
    W1 = din("W1", [MH, 128, KT, 128], BF16)   # [mo][p][kt][cols]; ln2_w folded
    W2 = din("W2", [MH, 128, HID], BF16)       # [k2][p][m]
    bqs = din("bqs", [HID], F32)
    bk = din("bk", [HID], F32)
    bv = din("bv", [HID], F32)
    bo = din("bo", [HID], F32)
    b1 = din("b1", [4 * HID], F32)
    b2 = din("b2", [HID], F32)

    y = nc.dram_tensor("y", [HID, QL], F32, kind="ExternalOutput").ap()
    if debug:
        d_ln1 = nc.dram_tensor("d_ln1", [128, KT, S], BF16,
                               kind="ExternalOutput").ap()
        d_K = nc.dram_tensor("d_K", [128, KT, S], BF16,
                             kind="ExternalOutput").ap()
        d_attn = nc.dram_tensor("d_attn", [128, KT, QL], BF16,
                                kind="ExternalOutput").ap()
        d_r = nc.dram_tensor("d_r", [128, KT, QL], BF16,
                             kind="ExternalOutput").ap()
        d_ln2 = nc.dram_tensor("d_ln2", [128, KT, QL], BF16,
                               kind="ExternalOutput").ap()

    def bcast(src_elem_ap, parts, n):
        return bass.AP(tensor=src_elem_ap.tensor, offset=src_elem_ap.offset,
                       ap=[[0, parts], [1, n]])

    from concourse import library_config

    with tile.TileContext(nc) as tc, ExitStack() as ctx:
        sb = ctx.enter_context(tc.tile_pool(name="sb", bufs=1))
        ps = ctx.enter_context(tc.tile_pool(name="ps", bufs=1, space="PSUM"))

        # partition_broadcast lives in the gpsimd "attn" ucode library
        nc.gpsimd.load_library(library_config.attn)

        def pst_tile(name, shape=(128, 1024)):
            return ps.tile(list(shape), F32, tag="s2", bufs=4, name=name,
                           padded_shape=[128, 1024])

        # ---------- constants ----------
        ones_bf = sb.tile([128, 1], BF16, tag="ones")
        nc.vector.memset(ones_bf, 1.0)
        par = sb.tile([128, 80], F32, tag="par")

        def load_cols(dst0, src, n):
            nc.gpsimd.dma_start(
                out=par[:, dst0:dst0 + n],
                in_=bass.AP(tensor=src.tensor, offset=src.offset,
                            ap=[[1, 128], [128, n]]))

        load_cols(24, bqs, KT)
        load_cols(30, bk, KT)
        load_cols(36, bo, KT)
        load_cols(42, b2, KT)
        load_cols(48, b1, MH)
        nc.vector.memset(par[:, 72:73], EPS)
        eps_t = par[0:1, 72:73]
        if not biases_zero:
            bv_b = sb.tile([128, HID], F32, tag="bv_b")
            nc.gpsimd.dma_start(out=bv_b, in_=bcast(bv[0], 128, HID))
        masks_sb = sb.tile([128, 2, 2, 128], BF16, tag="masks")
        nc.gpsimd.dma_start(out=masks_sb, in_=masks)

        # PE warmup: lift the HAM clock gate while input DMAs land
        warm = sb.tile([128, 512], BF16, tag="warm", bufs=1, name="warm")
        nc.vector.memset(warm, 0.0)
        wps = pst_tile("warmps", (1, 512))
        for i in range(8):
            nc.tensor.matmul(wps, ones_bf, warm, start=True, stop=True)

        # ---------- input load ----------
        xbT_sb = sb.tile([128, KT, S], BF16, tag="xbT")
        for c in range(S // 512):
            nc.sync.dma_start(out=xbT_sb[:, :, 512 * c:512 * c + 512],
                              in_=xbT[:, :, 512 * c:512 * c + 512])

        # ---------- LN helper: 1-lane stats + gpsimd partition broadcast ----
        def emit_ln(N, x_bf, out_bf, pfx):
            # 1-lane stats scratch shares the "lane" tag with attention's rr
            lnst = sb.tile([1, 4, 512], F32, tag="lane", bufs=1,
                           name=f"{pfx}lnst")
            for c in range(N // 512):
                off = 512 * c
                cs = slice(off, off + 512)
                s_ps = pst_tile(f"{pfx}s{c}", (1, 512))
                q_ps = pst_tile(f"{pfx}q{c}", (1, 512))
                for kt in range(KT):
                    nc.tensor.matmul(s_ps, ones_bf, x_bf[:, kt, cs],
                                     start=(kt == 0), stop=(kt == KT - 1))
                for kt in range(KT):
                    sqc = sb.tile([128, 512], BF16, tag="sq", bufs=2,
                                  name=f"{pfx}sq{c}_{kt}")
                    nc.scalar.activation(sqc, x_bf[:, kt, cs], Act.Square)
                    nc.tensor.matmul(q_ps, ones_bf, sqc,
                                     start=(kt == 0), stop=(kt == KT - 1))
                # 1-lane stats math: mu, m2, var, a32 (rstd)
                mu, m2, var, a32 = (lnst[:, i, :] for i in range(4))
                nc.vector.tensor_scalar_mul(mu, s_ps, 1.0 / HID)
                nc.vector.tensor_mul(m2, mu, mu)
                nc.vector.scalar_tensor_tensor(var, q_ps, 1.0 / HID, m2,
                                               Alu.mult, Alu.subtract)
                sd = m2  # reuse slot
                nc.scalar.activation(sd, var, Act.Sqrt, bias=eps_t, scale=1.0)
                nc.vector.reciprocal_approx_fast(out=a32, in_=sd)
                ac = sb.tile([1, 2, 512], BF16, tag="lnac", bufs=2,
                             name=f"{pfx}ac{c}")
                nc.vector.tensor_copy(ac[:, 0, :], a32)
                nc.vector.scalar_tensor_tensor(ac[:, 1, :], a32, -1.0, mu,
                                               Alu.mult, Alu.mult)  # -mu*rstd
                # broadcast across partitions on GpSimd
                acb = sb.tile([128, 2, 512], BF16, tag="lnacb", bufs=2,
                              name=f"{pfx}acb{c}")
                nc.gpsimd.partition_broadcast(acb[:, 0, :], ac[:, 0, :],
                                              channels=128)
                nc.gpsimd.partition_broadcast(acb[:, 1, :], ac[:, 1, :],
                                              channels=128)
                # apply: out = x*a + c  (bf16, 2x DVE mode)
                for kt in range(KT):
                    t0 = sb.tile([128, 512], BF16, tag="t0", bufs=2,
                                 name=f"{pfx}t0{c}_{kt}")
                    nc.vector.tensor_mul(t0, x_bf[:, kt, cs], acb[:, 0, :])
                    nc.vector.tensor_add(out_bf[:, kt, cs], t0, acb[:, 1, :])

        ln_bf = sb.tile([128, KT, S], BF16, tag="ln")
        emit_ln(S, xbT_sb, ln_bf, "l1")
        if debug:
            nc.scalar.dma_start(out=d_ln1, in_=ln_bf)

        # evacuation helper: psum -> sbuf (+ optional bias col)
        def evac(dst, src, bias_col=None):
            if biases_zero or bias_col is None:
                nc.scalar.copy(dst, src)
            else:
                nc.vector.tensor_scalar(dst, src, 1.0, bias_col,
                                        Alu.mult, Alu.add)

        # ---------- QKV projections (mo-outer, weights loaded once) ----------
        K_sb = sb.tile([128, KT, S], BF16, tag="K")
        Q_sb = sb.tile([128, KT, QL], BF16, tag="Q")
        # n-outer so chunk c's K matmuls start as soon as LN1 chunk c is
        # applied (keeps the PE warm through LN1). Wk is resident (one DMA
        # on the vector HWDGE queue, shares the tag of the later r_bf).
        Wk_sb = sb.tile([128, KT, HID], BF16, tag="r", name="Wk_sb")
        nc.sync.dma_start(out=Wk_sb, in_=Wk)
        for n in range(S // 512):
            cs = slice(512 * n, 512 * n + 512)
            for mo in range(KT):
                pst = pst_tile(f"kps{mo}_{n}", (128, 512))
                for kt in range(KT):
                    nc.tensor.matmul(pst,
                                     Wk_sb[:, kt, 128 * mo:128 * mo + 128],
                                     ln_bf[:, kt, cs],
                                     start=(kt == 0), stop=(kt == KT - 1))
                evac(K_sb[:, mo, cs], pst, par[:, 30 + mo:31 + mo])
        for mo in range(KT):
            wqt = sb.tile([128, KT, 128], BF16, tag="wk6", bufs=2,
                          name=f"wq{mo}")
            nc.sync.dma_start(out=wqt, in_=Wq[:, :, 128 * mo:128 * mo + 128])
            for n in range(QL // 512):
                cs = slice(512 * n, 512 * n + 512)
                pst = pst_tile(f"qps{mo}_{n}", (128, 512))
                for kt in range(KT):
                    nc.tensor.matmul(pst, wqt[:, kt, :], ln_bf[:, kt, cs],
                                     start=(kt == 0), stop=(kt == KT - 1))
                evac(Q_sb[:, mo, cs], pst, par[:, 24 + mo:25 + mo])

        V_sb = sb.tile([128, H, TT, 65], BF16, tag="V")
        for h in range(H):
            nc.vector.memset(V_sb[:, h, :, 64:65], 1.0)
        for fc in range(2):
            wvt = sb.tile([128, KT, 384], BF16, tag="wv", bufs=1, name=f"wv{fc}")
            nc.sync.dma_start(out=wvt,
                              in_=Wv[:, :, 384 * fc:384 * fc + 384])
            for tt in range(TT):
                pst = pst_tile(f"vps{tt}_{fc}", (128, 384))
                for kt in range(KT):
                    nc.tensor.matmul(pst, ln_bf[:, kt, 128 * tt:128 * tt + 128],
                                     wvt[:, kt, :],
                                     start=(kt == 0), stop=(kt == KT - 1))
                vdst = V_sb[:, 6 * fc:6 * fc + 6, tt, 0:64]
                if biases_zero:
                    nc.scalar.copy(vdst, pst.rearrange("p (h d) -> p h d", d=64))
                else:
                    nc.vector.tensor_tensor(
                        vdst, pst.rearrange("p (h d) -> p h d", d=64),
                        bv_b[:, 384 * fc:384 * fc + 384].rearrange(
                            "p (h d) -> p h d", d=64),
                        Alu.add)

        # prefetch Wo + a third of W1 during attention (gpsimd queue); the
        # remaining W1 streams per-mo during the MLP
        Wo_sb = sb.tile([128, KT, HID], BF16, tag="Wo")
        nc.sync.dma_start(out=Wo_sb, in_=Wo)
        W1_sb = sb.tile([128, 8, KT, 128], BF16, tag="W1")
        for mo in range(8):
            nc.sync.dma_start(out=W1_sb[:, mo, :, :], in_=W1[mo])

        # ---------- attention ----------
        attn_bf = sb.tile([128, KT, QL], BF16, tag="attn")
        # denominator reciprocal rows live at partition 64 (DVE lanes are
        # partition-aligned; O's ones-row is at partition 64)
        rr = sb.tile([65, 2, QL], F32, tag="lane", bufs=1, name="rr")
        # full 128 partitions: broadcast ucode may write all partitions of
        # the column range regardless of channels; give it its own rows
        rb = sb.tile([128, 2, QL], F32, tag="rbc", bufs=1, name="rb")
        for kt in range(KT):
            h0, h1 = 2 * kt, 2 * kt + 1
            O = {h0: pst_tile(f"o{h0}", (65, QL)),
                 h1: pst_tile(f"o{h1}", (65, QL))}

            def av(h, tile_t, e, c0, w):
                nc.tensor.matmul(O[h][:, c0:c0 + w],
                                 V_sb[:, h, tile_t, :], e,
                                 start=(tile_t == 0),
                                 stop=(tile_t == TT - 1),
                                 skip_group_check=True)

            pending = []
            for t in range(TT):
                q0 = 128 * (t % 8)
                midx = 0 if t < 8 else 1
                # chunks of the query span [q0:QL), each <=512 wide; merged
                # head-pair scores: h0 at [0:w], h1 at [512:512+w]
                if q0 < 512:
                    chunks = [(q0, 512 - q0, True), (512, 512, False)]
                else:
                    chunks = [(q0, QL - q0, True)]
                for (c0, w, diag) in chunks:
                    # h0 right-aligned at [512-w:512], h1 at [512:512+w] so
                    # the exp covers one contiguous span with no gap
                    S_m = pst_tile(f"sm{kt}_{t}_{c0}", (128, 1024))
                    for pr, co in ((slice(0, 64), 512 - w),
                                   (slice(64, 128), 512)):
                        nc.tensor.matmul(S_m[:, co:co + w],
                                         K_sb[pr, kt, 128 * t:128 * t + 128],
                                         Q_sb[pr, kt, c0:c0 + w],
                                         start=True, stop=True)
                    eS = sb.tile([128, 1024], BF16, tag="expS", bufs=4,
                                 name=f"es{kt}_{t}_{c0}")
                    nc.scalar.activation(eS[:, 512 - w:512 + w],
                                         S_m[:, 512 - w:512 + w], Act.Exp)
                    if diag:
                        for co in (512 - w, 512):
                            nc.vector.tensor_mul(
                                eS[:, co:co + 128], eS[:, co:co + 128],
                                masks_sb[:, midx, 0, :])
                    pending.append([(h0, eS[:, 512 - w:512], c0, w, t),
                                    (h1, eS[:, 512:512 + w], c0, w, t)])
                    if len(pending) > 2:
                        for (h, e, pc0, pw, pt) in pending.pop(0):
                            av(h, pt, e, pc0, pw)
            for grp in pending:
                for (h, e, pc0, pw, pt) in grp:
                    av(h, pt, e, pc0, pw)

            # Evacuate each head's full O (values + ones-row denominator) to
            # SBUF in ONE DVE op -> PSUM banks free 1.2us after the last AV.
            # Then: DMA the denominator row to partition 0, broadcast the raw
            # denominator on GpSimd (ucode reads partition 0 only), take the
            # reciprocal in place at partitions 0:64 (custom-DVE is broken at
            # non-zero base), and normalize from SBUF.
            nc.vector.tensor_copy(rr[0:65, 0, :], O[h0][0:65, :])
            nc.vector.tensor_copy(rr[0:65, 1, :], O[h1][0:65, :])
            for i in range(2):
                nc.sync.dma_start(out=rb[0:1, i, :], in_=rr[64:65, i, :])
                nc.gpsimd.partition_broadcast(rb[:, i, :], rb[0:1, i, :],
                                              channels=128)
                nc.vector.reciprocal_approx_fast(out=rb[0:64, i, :],
                                                 in_=rb[0:64, i, :])
            nc.vector.tensor_mul(attn_bf[0:64, kt, :], rr[0:64, 0, :],
                                 rb[0:64, 0, :])
            stg = sb.tile([64, QL], BF16, tag="stg", bufs=1, name=f"stg{kt}")
            nc.vector.tensor_mul(stg, rr[0:64, 1, :], rb[0:64, 1, :])
            nc.sync.dma_start(out=attn_bf[64:128, kt, :], in_=stg)

        # prefetch W2 (gpsimd queue) into space freed by Q
        if debug:
            nc.scalar.dma_start(out=d_K, in_=K_sb)
            nc.scalar.dma_start(out=d_attn, in_=attn_bf)
        W2a = sb.tile([128, 8, HID], BF16, tag="Q", name="W2a")
        for k2 in range(8):
            nc.sync.dma_start(out=W2a[:, k2, :], in_=W2[k2])

        # ---------- Wo + residual (bf16), token-chunk-outer, streamed Wo ----
        r_bf = sb.tile([128, KT, QL], BF16, tag="r")
        for n in range(QL // 512):
            cs = slice(512 * n, 512 * n + 512)
            for mo in range(KT):
                pst = pst_tile(f"ops{mo}_{n}", (128, 512))
                for kt in range(KT):
                    nc.tensor.matmul(pst,
                                     Wo_sb[:, kt, 128 * mo:128 * mo + 128],
                                     attn_bf[:, kt, cs],
                                     start=(kt == 0), stop=(kt == KT - 1))
                if biases_zero:
                    nc.vector.tensor_add(r_bf[:, mo, cs], pst,
                                         xbT_sb[:, mo, cs])
                else:
                    nc.vector.scalar_tensor_tensor(r_bf[:, mo, cs], pst,
                                                   par[:, 36 + mo:37 + mo],
                                                   xbT_sb[:, mo, cs],
                                                   Alu.add, Alu.add)

        # prefetch rest of W2 into space freed by attn / xbT
        if debug:
            nc.scalar.dma_start(out=d_r, in_=r_bf)
        W2b = sb.tile([128, 8, HID], BF16, tag="attn", name="W2b")
        for k2 in range(8):
            nc.sync.dma_start(out=W2b[:, k2, :], in_=W2[8 + k2])
        W2c = sb.tile([128, 8, HID], BF16, tag="xbT", name="W2c")
        for k2 in range(8):
            nc.sync.dma_start(out=W2c[:, k2, :], in_=W2[16 + k2])
        W2t = (W2a, W2b, W2c)

        ln2_bf = sb.tile([128, KT, QL], BF16, tag="V", name="ln2")
        emit_ln(QL, r_bf, ln2_bf, "l2")

        # ---------- MLP: W1 mo-outer (half resident), then W2 per cs-half ----
        if debug:
            nc.scalar.dma_start(out=d_ln2, in_=ln2_bf)
        # keep-warm: dummy matmuls tied to LN2 chunk-0 slices keep the PE
        # clock hot across the LN2 latency bubble before the MLP
        wps2 = pst_tile("warm2", (1, 512))
        for kt in range(KT):
            nc.tensor.matmul(wps2, ones_bf, ln2_bf[:, kt, 0:512],
                             start=True, stop=True)
        g0 = sb.tile([128, 12, QL], BF16, tag="ln", name="g0")
        g1 = sb.tile([128, 12, QL], BF16, tag="K", name="g1")
        for mo in range(MH):
            gt = g0 if mo < 12 else g1
            if mo < 8:
                w1t = W1_sb[:, mo, :, :]
            else:
                w1t = sb.tile([128, KT, 128], BF16, tag="wk6", bufs=2,
                              name=f"w1s{mo}")
                nc.sync.dma_start(out=w1t, in_=W1[mo])
            for n in range(QL // 512):
                cs = slice(512 * n, 512 * n + 512)
                pst = pst_tile(f"h1ps{mo}_{n}", (128, 512))
                for kt in range(KT):
                    nc.tensor.matmul(pst, w1t[:, kt, :],
                                     ln2_bf[:, kt, cs],
                                     start=(kt == 0), stop=(kt == KT - 1))
                if biases_zero:
                    nc.scalar.activation(gt[:, mo % 12, cs], pst, Act.Gelu)
                else:
                    nc.scalar.activation(gt[:, mo % 12, cs], pst, Act.Gelu,
                                         bias=par[:, 48 + mo:49 + mo],
                                         scale=1.0)
        for n in range(QL // 512):
            cs = slice(512 * n, 512 * n + 512)
            psts = [pst_tile(f"yps{n}_{i}") for i in range(3)]
            for k2 in range(MH):
                gt = g0 if k2 < 12 else g1
                for mo in range(KT):
                    nc.tensor.matmul(
                        psts[mo // 2][:, 512 * (mo % 2):512 * (mo % 2) + 512],
                        W2t[k2 // 8][:, k2 % 8, 128 * mo:128 * mo + 128],
                        gt[:, k2 % 12, cs],
                        start=(k2 == 0), stop=(k2 == MH - 1))
            for mo in range(KT):
                pslice = psts[mo // 2][:, 512 * (mo % 2):512 * (mo % 2) + 512]
                yst = sb.tile([128, 512], F32, tag="yst", bufs=2,
                              name=f"yst{n}_{mo}")
                if biases_zero:
                    nc.vector.tensor_add(yst, pslice, r_bf[:, mo, cs])
                else:
                    nc.vector.scalar_tensor_tensor(yst, pslice,
                                                   par[:, 42 + mo:43 + mo],
                                                   r_bf[:, mo, cs],
                                                   Alu.add, Alu.add)
                nc.sync.dma_start(out=y[128 * mo:128 * mo + 128, cs], in_=yst)

    nc.compile()
    return nc


def _get_program(biases_zero):
    key = ("nc", biases_zero)
    if key not in _CACHE:
        _CACHE[key] = _build_program(biases_zero)
    return _CACHE[key]


def _prep_in_maps(inputs):
    import ml_dtypes
    bf = ml_dtypes.bfloat16
    f32 = np.float32

    x = np.ascontiguousarray(np.asarray(inputs["x"], dtype=f32))
    ln1w = np.asarray(inputs["ln1_w"], f32)
    ln1b = np.asarray(inputs["ln1_b"], f32)
    ln2w = np.asarray(inputs["ln2_w"], f32)
    ln2b = np.asarray(inputs["ln2_b"], f32)
    Wq = np.asarray(inputs["Wq"], f32)
    Wk = np.asarray(inputs["Wk"], f32)
    Wv = np.asarray(inputs["Wv"], f32)
    Wo = np.asarray(inputs["Wo"], f32)
    W1 = np.asarray(inputs["W1"], f32)
    W2 = np.asarray(inputs["W2"], f32)
    # fold LN gains into consumer weights; LN bias contribution into proj
    # biases; fold the 1/sqrt(D) score scale into Wq
    Wq_f = ln1w[:, None] * Wq * np.float32(1.0 / np.sqrt(D))
    Wk_f = ln1w[:, None] * Wk
    Wv_f = ln1w[:, None] * Wv
    W1_f = ln2w[:, None] * W1
    bq_e = (Wq.T @ ln1b + np.asarray(inputs["bq"], f32)) / np.float32(np.sqrt(D))
    bk_e = Wk.T @ ln1b + np.asarray(inputs["bk"], f32)
    bv_e = Wv.T @ ln1b + np.asarray(inputs["bv"], f32)
    b1_e = W1.T @ ln2b + np.asarray(inputs["b1"], f32)
    bo_e = np.asarray(inputs["bo"], f32)
    b2_e = np.asarray(inputs["b2"], f32)
    biases_zero = bool(
        all(np.all(v == 0) for v in (bq_e, bk_e, bv_e, b1_e, bo_e, b2_e)))

    def pack_kp(W):  # [HID, M] -> [128, KT, M] with row k*128+p -> [p, k]
        M = W.shape[1]
        return np.ascontiguousarray(
            W.reshape(KT, 128, M).transpose(1, 0, 2).astype(bf))

    W1p = pack_kp(W1_f)                       # [128, KT, 3072]
    W1p = np.ascontiguousarray(
        W1p.reshape(128, KT, MH, 128).transpose(2, 0, 1, 3))  # [mo][p][kt][128]
    W2p = np.ascontiguousarray(
        W2.reshape(MH, 128, HID).astype(bf))  # [k2][p][m]

    shared = {
        "Wq": pack_kp(Wq_f),
        "Wk": pack_kp(Wk_f),
        "Wv": pack_kp(Wv_f),
        "Wo": pack_kp(Wo),
        "W1": W1p,
        "W2": W2p,
        "bqs": bq_e, "bk": bk_e, "bv": bv_e, "bo": bo_e,
        "b1": b1_e, "b2": b2_e,
    }

    in_maps = []
    qcols_all = []
    for c in range(N_CORES):
        b, p = c // 2, c % 2
        qcols = np.concatenate(
            [np.arange(128 * (2 * j + p), 128 * (2 * j + p) + 128)
             for j in range(8)])
        ocols = np.concatenate(
            [np.arange(128 * (2 * j + 1 - p), 128 * (2 * j + 1 - p) + 128)
             for j in range(8)])
        qcols_all.append(qcols)
        xp = np.concatenate([x[b][qcols], x[b][ocols]], axis=0)  # [S, HID]
        xbT = np.ascontiguousarray(
            xp.T.reshape(KT, 128, S).transpose(1, 0, 2).astype(bf))
        # masks: [tri (own-parity diagonal) | flat (other-parity first block)]
        # duplicated along a head-rep axis so one DVE op masks both heads
        m = np.zeros((2, 128, 128), np.float32)
        kk = np.arange(128)[:, None]
        qq = np.arange(128)[None, :]
        m[0] = (kk <= qq).astype(np.float32)
        m[1] = 0.0 if p == 0 else 1.0
        m2 = np.stack([m, m], axis=1)                   # [2, 2, 128, 128]
        mperm = np.ascontiguousarray(m2.transpose(2, 0, 1, 3))  # [128,2,2,128]
        im = dict(shared)
        im["xbT"] = xbT
        im["masks"] = mperm.astype(bf)
        in_maps.append(im)
    return in_maps, qcols_all, biases_zero


def kernel(**inputs):
    import sys, types
    if "antenv.axon_hooks" not in sys.modules:
        try:
            sys.path.insert(0, "/root/.axon_site")
            from trn_agent_boot.trn_boot import _ntff_profile_via_ctypes
            hook = _ntff_profile_via_ctypes("/opt/axon/libaxon_pjrt.so")
            mod = types.ModuleType("antenv.axon_hooks")
            mod.get_axon_ntff_profile_hook = lambda: hook
            mod.set_axon_ntff_profile_hook = lambda h: None
            import antenv  # noqa: F401
            sys.modules["antenv.axon_hooks"] = mod
        except Exception:
            pass

    from concourse.bass_utils import run_bass_kernel_spmd

    in_maps, qcols_all, biases_zero = _prep_in_maps(inputs)
    nc = _get_program(biases_zero)
    res = run_bass_kernel_spmd(nc, in_maps, core_ids=list(range(N_CORES)))
    out = np.zeros((B, S, HID), np.float32)
    for c in range(N_CORES):
        out[c // 2, qcols_all[c], :] = res.results[c]["y"].T
    return out
